# revision 1
# baseline (speedup 1.0000x reference)
import numpy as np

# GPT-style model dims (hardcoded per problem spec nn_LLM_773094113519)
L, B, S, D, H, V, F = 4, 2, 2048, 1024, 16, 50257, 4096
DH = D // H
M = B * S                      # 4096 flattened tokens
NCORES = 8
PERCORE = -(-V // NCORES)      # 6283 vocab cols per core (last core ragged)
NPAD = 6656                    # 13 * 512, padded per-core col count


def _ln(x, w, b):
    m = x.mean(-1, keepdims=True, dtype=np.float32)
    v = ((x - m) ** 2).mean(-1, keepdims=True, dtype=np.float32)
    return ((x - m) / np.sqrt(v + 1e-5) * w + b).astype(np.float32)


def _rope(x):
    dh = x.shape[-1]
    inv = 1.0 / (10000.0 ** (np.arange(0, dh, 2, dtype=np.float32) / dh))
    t = np.arange(x.shape[-2], dtype=np.float32)
    fr = t[:, None] * inv[None, :]
    emb = np.concatenate([fr, fr], axis=-1)
    cos, sin = np.cos(emb).astype(np.float32), np.sin(emb).astype(np.float32)
    half = dh // 2
    x1, x2 = x[..., :half], x[..., half:]
    rot = np.concatenate([-x2, x1], axis=-1)
    return (x * cos + rot * sin).astype(np.float32)


def _gelu(x):
    try:
        from scipy.special import erf
        return (x * 0.5 * (1.0 + erf(x / np.sqrt(2.0).astype(np.float32)))).astype(np.float32)
    except Exception:
        import jax
        import jax.numpy as jnp
        with jax.default_device(jax.devices("cpu")[0]):
            return np.asarray(jax.nn.gelu(jnp.asarray(x), approximate=False))


def _softmax_lastdim(x):
    mx = x.max(-1, keepdims=True)
    e = np.exp(x - mx)
    return e / e.sum(-1, keepdims=True, dtype=np.float32)


def _forward_layers(tokens, pos_emb, word_emb, ln1_w, ln1_b, wq, bq, wk, bk,
                    wv, bv, wo, bo, ln2_w, ln2_b, w1, b1, w2, b2,
                    post_w, post_b, lnf_w, lnf_b):
    x = (word_emb[tokens] + pos_emb[None, :S, :]).reshape(M, D)
    x = x.astype(np.float32)
    scale = np.float32(1.0 / np.sqrt(DH))
    neg = np.float32(-1e9)
    mask = np.tril(np.ones((S, S), dtype=bool))
    for i in range(L):
        h = _ln(x, ln1_w[i], ln1_b[i])
        hf = h
        q = (hf @ wq[i] + bq[i]).reshape(B, S, H, DH).transpose(0, 2, 1, 3)
        k = (hf @ wk[i] + bk[i]).reshape(B, S, H, DH).transpose(0, 2, 1, 3)
        v = (hf @ wv[i] + bv[i]).reshape(B, S, H, DH).transpose(0, 2, 1, 3)
        q, k = _rope(q), _rope(k)
        o = np.empty((B, H, S, DH), np.float32)
        for b_ in range(B):
            for h_ in range(H):
                sc = (q[b_, h_] @ k[b_, h_].T) * scale
                sc = np.where(mask, sc, neg).astype(np.float32)
                att = _softmax_lastdim(sc)
                o[b_, h_] = att @ v[b_, h_]
        o = o.transpose(0, 2, 1, 3).reshape(M, D)
        x = (x + o @ wo[i] + bo[i]).astype(np.float32)
        h2 = _ln(x, ln2_w[i], ln2_b[i])
        x = (x + _gelu(h2 @ w1[i] + b1[i]) @ w2[i] + b2[i]).astype(np.float32)
        if i == L - 1:
            x = _ln(x, post_w, post_b)
    x = _ln(x, lnf_w, lnf_b)
    return x.astype(np.float32)


def _bass_head_logits(x, head_w):
    """x: [M, D] f32, head_w: [D, V] f32 -> logits [M, V] via 8-core
    column-sharded matmul on trn2."""
    from concourse import bass, bacc, tile, bass_utils
    import concourse.mybir as mybir

    KT = D // 128      # 8 k-tiles of 128
    NT = NPAD // 512   # 13 n-tiles of 512
    MT = M // 128      # 32 m-tiles of 128

    nc = bacc.Bacc("TRN2", target_bir_lowering=False, debug=False,
                   num_devices=NCORES)
    xT_d = nc.dram_tensor("xT", (KT, 128, M), mybir.dt.float32,
                          kind="ExternalInput").ap()
    w_d = nc.dram_tensor("w", (KT, 128, NPAD), mybir.dt.float32,
                         kind="ExternalInput").ap()
    out_d = nc.dram_tensor("out", (M, NPAD), mybir.dt.float32,
                           kind="ExternalOutput").ap()

    with tile.TileContext(nc) as tc:
        with tc.tile_pool(name="xpool", bufs=1) as xpool, \
             tc.tile_pool(name="wpool", bufs=2) as wpool, \
             tc.tile_pool(name="opool", bufs=4) as opool, \
             tc.tile_pool(name="psum", bufs=4, space=bass.MemorySpace.PSUM) as pp:
            xT = xpool.tile([128, KT * M], mybir.dt.float32)
            for k in range(KT):
                nc.sync.dma_start(xT[:, k * M:(k + 1) * M], xT_d[k])
            for n in range(NT):
                wt = wpool.tile([128, KT * 512], mybir.dt.float32)
                for k in range(KT):
                    nc.sync.dma_start(wt[:, k * 512:(k + 1) * 512],
                                      w_d[k, :, n * 512:(n + 1) * 512])
                for m in range(MT):
                    ps = pp.tile([128, 512], mybir.dt.float32)
                    for k in range(KT):
                        nc.tensor.matmul(
                            ps[:],
                            xT[:, k * M + m * 128: k * M + (m + 1) * 128],
                            wt[:, k * 512:(k + 1) * 512],
                            start=(k == 0), stop=(k == KT - 1))
                    ot = opool.tile([128, 512], mybir.dt.float32)
                    nc.vector.tensor_copy(ot[:], ps[:])
                    nc.sync.dma_start(
                        out_d[m * 128:(m + 1) * 128, n * 512:(n + 1) * 512],
                        ot[:])
    nc.compile()

    xT_np = np.ascontiguousarray(x.T.reshape(KT, 128, M))
    in_maps = []
    for c in range(NCORES):
        lo = c * PERCORE
        hi = min(lo + PERCORE, V)
        ws = np.zeros((D, NPAD), np.float32)
        ws[:, :hi - lo] = head_w[:, lo:hi]
        in_maps.append({"xT": xT_np,
                        "w": np.ascontiguousarray(ws.reshape(KT, 128, NPAD))})
    res = bass_utils.run_bass_kernel_spmd(nc, in_maps,
                                          core_ids=list(range(NCORES)))
    shards = []
    for c in range(NCORES):
        lo = c * PERCORE
        hi = min(lo + PERCORE, V)
        shards.append(res.results[c]["out"][:, :hi - lo])
    return np.concatenate(shards, axis=1)


def kernel(tokens, targets, word_emb, pos_emb, ln1_w, ln1_b, wq, bq, wk, bk,
           wv, bv, wo, bo, ln2_w, ln2_b, w1, b1, w2, b2, post_w, post_b,
           lnf_w, lnf_b, head_w):
    tokens = np.asarray(tokens)
    targets = np.asarray(targets)
    f32 = lambda a: np.asarray(a, dtype=np.float32)
    x = _forward_layers(tokens, f32(pos_emb), f32(word_emb), f32(ln1_w),
                        f32(ln1_b), f32(wq), f32(bq), f32(wk), f32(bk),
                        f32(wv), f32(bv), f32(wo), f32(bo), f32(ln2_w),
                        f32(ln2_b), f32(w1), f32(b1), f32(w2), f32(b2),
                        f32(post_w), f32(post_b), f32(lnf_w), f32(lnf_b))
    try:
        logits = _bass_head_logits(x, f32(head_w))
    except Exception as e:
        import traceback
        traceback.print_exc()
        logits = x @ f32(head_w)
    mx = logits.max(-1, keepdims=True)
    lse = (mx + np.log(np.exp(logits - mx).sum(-1, keepdims=True,
                                               dtype=np.float32))).astype(np.float32)
    tgt = targets.reshape(M).astype(np.int64)
    picked = logits[np.arange(M), tgt]
    nll = -(picked - lse[:, 0])
    return np.float32(nll.mean(dtype=np.float32))



# revision 2
# speedup vs baseline: 1.4990x; 1.4990x over previous
"""Tensor-parallel GPT forward pass on 8 trn2 NeuronCores (Bass/Tile).

Sharding: heads (16 -> 2/core) for attention; MLP hidden (4096 -> 512/core);
vocab head (50257 -> 6283/core). LayerNorms run on token shards (512
tokens/core); h is AllGather'd (bf16) before the sharded matmuls and partial
outputs are ReduceScatter'd back onto token shards. Residual stream stays f32
on the owning core. Embedding lookup and the final NLL reduction run on host.
"""
import numpy as np
import ml_dtypes

L, B, S, D, H, V, FF = 4, 2, 2048, 1024, 16, 50257, 4096
DH = 64
M = B * S                    # 4096 tokens
NC = 8
MSH = M // NC                # 512 tokens / core
DSH = 2 * DH                 # 128 qkv dims / core (2 heads)
FSH = FF // NC               # 512 mlp dims / core
PERV = -(-V // NC)           # 6283 vocab cols / core
VPAD = 6656                  # 13 * 512
NKT = D // 128               # 8 k-tiles over D
NMT = M // 128               # 32 m-tiles over tokens
NCH = M // 512               # 8 512-token chunks
NVC = VPAD // 512            # 13 vocab chunks / core

bf16 = ml_dtypes.bfloat16

_CACHE = {}


def _build():
    from concourse import bass, bacc, tile
    import concourse.mybir as mybir

    F32, BF16 = mybir.dt.float32, mybir.dt.bfloat16
    AX, ALU = mybir.AxisListType, mybir.AluOpType
    ACT = mybir.ActivationFunctionType

    nc = bacc.Bacc("TRN2", target_bir_lowering=False, debug=False, num_devices=NC)

    def din(name, shape, dt=F32):
        return nc.dram_tensor(name, shape, dt, kind="ExternalInput").ap()

    X0 = din("X0", (MSH, D))
    WQ = din("WQ", (L, D, DSH), BF16)
    WK = din("WK", (L, D, DSH), BF16)
    WV = din("WV", (L, D, DSH), BF16)
    WO = din("WO", (L, DSH, D), BF16)
    W1 = din("W1", (L, D, FSH), BF16)
    W2 = din("W2", (L, FSH, D), BF16)
    BQ = din("BQ", (L, DSH))
    BK = din("BK", (L, DSH))
    BV = din("BV", (L, DSH))
    B1 = din("B1", (L, FSH))
    LNP = din("LNP", (L, 6, D))        # ln1w ln1b ln2w ln2b bo b2
    LNF = din("LNF", (4, D))           # postw postb lnfw lnfb
    COS = din("COS", (DH, M), BF16)
    SINPM = din("SINPM", (DH, M), BF16)
    HWT = din("HW", (D, VPAD), BF16)

    XF = nc.dram_tensor("XF", (MSH, D), BF16, kind="ExternalOutput").ap()
    SE = nc.dram_tensor("SE", (128, NMT), F32, kind="ExternalOutput").ap()

    RG = [list(range(NC))]

    with tile.TileContext(nc) as tc:
        with tc.tile_pool(name="persist", bufs=1) as pst, \
             tc.tile_pool(name="dram", bufs=1, space="DRAM") as dram:
            # residual stream f32 [128, 4*1024]; m-tile m -> cols m*D..(m+1)*D
            xc = pst.tile([128, 4 * D], F32)
            for m in range(4):
                nc.sync.dma_start(xc[:, m * D:(m + 1) * D], X0[m * 128:(m + 1) * 128, :])
            # rope tables [128, M] (two heads stacked = same 64 rows twice)
            cos = pst.tile([128, M], BF16)
            sinpm = pst.tile([128, M], BF16)
            nc.sync.dma_start(cos[0:64, :], COS)
            nc.sync.dma_start(sinpm[0:64, :], SINPM)
            nc.sync.dma_start(cos[64:128, :], cos[0:64, :])
            nc.sync.dma_start(sinpm[64:128, :], sinpm[0:64, :])
            ones64 = pst.tile([1, 64], F32)
            nc.vector.memset(ones64[:], 1.0)
            ones128 = pst.tile([128, 1], BF16)
            nc.vector.memset(ones128[:], 1.0)
            epsb = pst.tile([128, 1], F32)
            nc.vector.memset(epsb[:], 1e-5)
            # identity (bf16) for PE transposes: 1.0 on diagonal
            ident = pst.tile([128, 128], BF16)
            nc.gpsimd.memset(ident[:], 1.0)
            nc.gpsimd.affine_select(ident[:], ident[:], pattern=[[1, 128]],
                                    compare_op=ALU.is_equal, fill=0.0,
                                    base=0, channel_multiplier=-1)

            def layer_norm(pool, w_rep, b_rep, src_ap_fn, dst_ap_fn, dst_dtype_f32=False):
                """LN over 4 m-tiles; src/dst_ap_fn(m) -> AP [128, D]."""
                for m in range(4):
                    src = src_ap_fn(m)
                    sums = pool.tile([128, 1], F32, tag="ln_sums")
                    sq = pool.tile([128, D], F32, tag="ln_sq")
                    sumsq = pool.tile([128, 1], F32, tag="ln_sumsq")
                    nc.vector.tensor_reduce(sums[:], src, axis=AX.X, op=ALU.add)
                    nc.scalar.activation(sq[:], src, ACT.Square, accum_out=sumsq[:])
                    mean = pool.tile([128, 1], F32, tag="ln_mean")
                    nc.vector.tensor_scalar(mean[:], sums[:], 1.0 / D, None, op0=ALU.mult)
                    m2 = pool.tile([128, 1], F32, tag="ln_m2")
                    nc.vector.tensor_scalar(m2[:], mean[:], mean[:], None, op0=ALU.mult)
                    var = pool.tile([128, 1], F32, tag="ln_var")
                    nc.vector.tensor_scalar(var[:], sumsq[:], 1.0 / D, m2[:], op0=ALU.mult, op1=ALU.subtract)
                    sd = pool.tile([128, 1], F32, tag="ln_sd")
                    nc.scalar.activation(sd[:], var[:], ACT.Sqrt, bias=epsb[:])
                    rstd = pool.tile([128, 1], F32, tag="ln_rstd")
                    nc.vector.reciprocal(rstd[:], sd[:])
                    mrstd = pool.tile([128, 1], F32, tag="ln_mrstd")
                    nc.vector.tensor_scalar(mrstd[:], mean[:], rstd[:], None, op0=ALU.mult)
                    norm = pool.tile([128, D], F32, tag="ln_norm")
                    nc.vector.tensor_scalar(norm[:], src, rstd[:], mrstd[:], op0=ALU.mult, op1=ALU.subtract)
                    # affine: out = norm * w + b
                    nw = pool.tile([128, D], F32, tag="ln_nw")
                    nc.vector.tensor_tensor(nw[:], norm[:], w_rep, op=ALU.mult)
                    out = dst_ap_fn(m)
                    nc.vector.tensor_tensor(out, nw[:], b_rep, op=ALU.add)

            for l in range(L):
                with tc.tile_pool(name=f"wl{l}", bufs=1) as wl, \
                     tc.tile_pool(name=f"rep{l}", bufs=1) as rep, \
                     tc.tile_pool(name=f"scr{l}", bufs=1) as scr:
                    # --- replicated per-layer params ---
                    reps = []
                    for j in range(6):
                        row = scr.tile([1, D], F32, tag="lnrow")
                        nc.sync.dma_start(row[:], LNP[l, j:j + 1, :])
                        r = rep.tile([128, D], F32, tag=f"rep{j}")
                        nc.gpsimd.partition_broadcast(r[:], row[:])
                        reps.append(r)
                    ln1w, ln1b, ln2w, ln2b, bo_r, b2_r = reps
                    # qkv biases as [128,1] f32
                    bq = rep.tile([128, 1], F32, tag="bq")
                    bk = rep.tile([128, 1], F32, tag="bk")
                    bv = rep.tile([128, 1], F32, tag="bv")
                    nc.sync.dma_start(bq[:], BQ[l, :].rearrange("(p o) -> p o", o=1))
                    nc.sync.dma_start(bk[:], BK[l, :].rearrange("(p o) -> p o", o=1))
                    nc.sync.dma_start(bv[:], BV[l, :].rearrange("(p o) -> p o", o=1))
                    b1t = rep.tile([128, 4], F32, tag="b1")  # 4 f-tiles of FSH
                    nc.sync.dma_start(b1t[:], B1[l, :].rearrange("(f p) -> p f", p=128))
                    # --- weights ---
                    wq = wl.tile([128, NKT * DSH], BF16, tag="wq")
                    wk = wl.tile([128, NKT * DSH], BF16, tag="wk")
                    wv = wl.tile([128, NKT * DSH], BF16, tag="wv")
                    for k in range(NKT):
                        nc.sync.dma_start(wq[:, k * DSH:(k + 1) * DSH], WQ[l, k * 128:(k + 1) * 128, :])
                        nc.sync.dma_start(wk[:, k * DSH:(k + 1) * DSH], WK[l, k * 128:(k + 1) * 128, :])
                        nc.sync.dma_start(wv[:, k * DSH:(k + 1) * DSH], WV[l, k * 128:(k + 1) * 128, :])
                    woA = wl.tile([64, D], BF16, tag="woA")
                    woB = wl.tile([64, D], BF16, tag="woB")
                    nc.sync.dma_start(woA[:], WO[l, 0:64, :])
                    nc.sync.dma_start(woB[:], WO[l, 64:128, :])
                    w1 = wl.tile([128, NKT * FSH], BF16, tag="w1")
                    for k in range(NKT):
                        nc.sync.dma_start(w1[:, k * FSH:(k + 1) * FSH], W1[l, k * 128:(k + 1) * 128, :])
                    w2 = wl.tile([128, 4 * D], BF16, tag="w2")
                    for k in range(4):
                        nc.sync.dma_start(w2[:, k * D:(k + 1) * D], W2[l, k * 128:(k + 1) * 128, :])

                    # === LN1 -> h (bf16) -> AG ===
                    h16 = scr.tile([128, 4 * D], BF16, tag="h16")
                    with tc.tile_pool(name=f"ln{l}a", bufs=2) as lnp_:
                        layer_norm(lnp_, ln1w[:], ln1b[:],
                                   lambda m: xc[:, m * D:(m + 1) * D],
                                   lambda m: h16[:, m * D:(m + 1) * D])
                    agin = dram.tile([MSH, D], BF16, tag="agin")
                    for m in range(4):
                        nc.sync.dma_start(agin[m * 128:(m + 1) * 128, :], h16[:, m * D:(m + 1) * D])
                    agout = dram.tile([M, D], BF16, tag="agout")
                    nc.gpsimd.collective_compute("AllGather", mybir.AluOpType.bypass,
                                                 replica_groups=RG, ins=[agin[:]], outs=[agout[:]])

                    # === qkv projections (feature-major outputs [DSH, M]) ===
                    qT = scr.tile([128, M], BF16, tag="qT")
                    kT = scr.tile([128, M], BF16, tag="kT")
                    vT = scr.tile([128, M], BF16, tag="vT")
                    with tc.tile_pool(name=f"hc{l}a", bufs=3) as hc, \
                         tc.tile_pool(name=f"ps{l}a", bufs=4, space=bass.MemorySpace.PSUM) as pq:
                        for c in range(NCH):
                            hts = []
                            for k in range(NKT):
                                ht = hc.tile([128, 512], BF16, tag=f"ht{k}")
                                nc.sync.dma_start_transpose(ht[:], agout[c * 512:(c + 1) * 512, k * 128:(k + 1) * 128])
                                hts.append(ht)
                            for (w_sb, bias_sb, outT) in ((wq, bq, qT), (wk, bk, kT), (wv, bv, vT)):
                                ps = pq.tile([128, 512], F32, tag="psqkv")
                                for k in range(NKT):
                                    nc.tensor.matmul(ps[:], w_sb[:, k * DSH:(k + 1) * DSH], hts[k][:],
                                                     start=(k == 0), stop=(k == NKT - 1))
                                nc.scalar.activation(outT[:, c * 512:(c + 1) * 512], ps[:], ACT.Identity, bias=bias_sb[:])

                    # === RoPE on qT, kT (in place) ===
                    with tc.tile_pool(name=f"rope{l}", bufs=1) as rp:
                        for src in (qT, kT):
                            shuf = rp.tile([128, M], BF16, tag="shuf")
                            nc.sync.dma_start(shuf[0:32, :], src[32:64, :])
                            nc.sync.dma_start(shuf[32:64, :], src[0:32, :])
                            nc.sync.dma_start(shuf[64:96, :], src[96:128, :])
                            nc.sync.dma_start(shuf[96:128, :], src[64:96, :])
                            t1 = rp.tile([128, M], BF16, tag="ropet1")
                            nc.vector.tensor_tensor(t1[:], src[:], cos[:], op=ALU.mult)
                            t2 = rp.tile([128, M], BF16, tag="ropet2")
                            nc.vector.tensor_tensor(t2[:], shuf[:], sinpm[:], op=ALU.mult)
                            nc.vector.tensor_tensor(src[:], t1[:], t2[:], op=ALU.add)
                    qTr, kTr = qT, kT

                    # === vT -> token-major v per head ===
                    vA = scr.tile([128, 32 * 64], BF16, tag="vA")
                    vB = scr.tile([128, 32 * 64], BF16, tag="vB")
                    with tc.tile_pool(name=f"vtr{l}", bufs=4, space=bass.MemorySpace.PSUM) as pv:
                        for blk in range(32):
                            pt = pv.tile([128, 128], BF16, tag="ptr")
                            nc.tensor.transpose(pt[:], vT[:, blk * 128:(blk + 1) * 128], ident[:])
                            nc.vector.tensor_copy(vA[:, blk * 64:(blk + 1) * 64], pt[:, 0:64])
                            nc.vector.tensor_copy(vB[:, blk * 64:(blk + 1) * 64], pt[:, 64:128])

                    # === attention ===
                    oA = scr.tile([64, M], BF16, tag="oA")
                    oB = scr.tile([64, M], BF16, tag="oB")
                    with tc.tile_pool(name=f"att{l}", bufs=3) as ap_, \
                         tc.tile_pool(name=f"aps{l}", bufs=2, space=bass.MemorySpace.PSUM) as ps_s, \
                         tc.tile_pool(name=f"apo{l}", bufs=2, space=bass.MemorySpace.PSUM) as ps_o, \
                         tc.tile_pool(name=f"apm{l}", bufs=2, space=bass.MemorySpace.PSUM) as ps_m, \
                         tc.tile_pool(name=f"apr{l}", bufs=2, space=bass.MemorySpace.PSUM) as ps_r:
                        for b in range(B):
                            for hh in range(2):
                                pb = hh * 64
                                vh = vA if hh == 0 else vB
                                oh = oA if hh == 0 else oB
                                for c in range(4):
                                    q_ap = qTr[pb:pb + 64, b * S + c * 512: b * S + (c + 1) * 512]
                                    po = ps_o.tile([64, 512], F32, tag="po")
                                    psum_ = ps_m.tile([1, 512], F32, tag="psum_")
                                    nkt = 4 * (c + 1)
                                    for kt in range(nkt):
                                        pscr = ps_s.tile([128, 512], F32, tag="pscr")
                                        k_ap = kTr[pb:pb + 64, b * S + kt * 128: b * S + (kt + 1) * 128]
                                        nc.tensor.matmul(pscr[:], k_ap, q_ap, start=True, stop=True)
                                        est = ap_.tile([128, 512], BF16, tag="est")
                                        nc.scalar.activation(est[:], pscr[:], ACT.Exp, scale=0.125)
                                        if kt >= 4 * c:
                                            # zero keys j > q: key j = kt*128+p, query q = c*512+f
                                            # keep iff f - p - off >= 0 over cols [0, off+128)
                                            off = kt * 128 - c * 512
                                            nc.gpsimd.affine_select(
                                                est[:, 0:off + 128], est[:, 0:off + 128],
                                                pattern=[[1, off + 128]], compare_op=ALU.is_ge,
                                                fill=0.0, base=-off, channel_multiplier=-1)
                                        nc.tensor.matmul(po[:], vh[:, (b * 16 + kt) * 64:(b * 16 + kt + 1) * 64],
                                                         est[:], start=(kt == 0), stop=(kt == nkt - 1))
                                        nc.tensor.matmul(psum_[:], ones128[:], est[:],
                                                         start=(kt == 0), stop=(kt == nkt - 1))
                                    rec = ap_.tile([1, 512], F32, tag="rec")
                                    nc.vector.reciprocal(rec[:], psum_[:])
                                    prr = ps_r.tile([64, 512], F32, tag="prr")
                                    nc.tensor.matmul(prr[:], ones64[:], rec[:], start=True, stop=True)
                                    pos = ap_.tile([64, 512], F32, tag="pos")
                                    nc.scalar.activation(pos[:], po[:], ACT.Copy)
                                    nc.vector.tensor_tensor(oh[:, b * S + c * 512: b * S + (c + 1) * 512],
                                                            pos[:], prr[:], op=ALU.mult)

                    # === wo projection -> partial [M, D] -> RS -> residual ===
                    rsin = dram.tile([M, D], BF16, tag="rsin")
                    with tc.tile_pool(name=f"wo{l}", bufs=3) as wop, \
                         tc.tile_pool(name=f"wops{l}", bufs=4, space=bass.MemorySpace.PSUM) as pwo:
                        for m in range(NMT):
                            att_sb = wop.tile([128, D], BF16, tag="attsb")
                            for n in range(2):
                                ps1 = pwo.tile([128, 512], F32, tag="pswo1")
                                ps2 = pwo.tile([128, 512], F32, tag="pswo2")
                                nc.tensor.matmul(ps1[:], oA[:, m * 128:(m + 1) * 128],
                                                 woA[:, n * 512:(n + 1) * 512], start=True, stop=True)
                                nc.tensor.matmul(ps2[:], oB[:, m * 128:(m + 1) * 128],
                                                 woB[:, n * 512:(n + 1) * 512], start=True, stop=True)
                                p1s = wop.tile([128, 512], F32, tag="p1s")
                                nc.scalar.activation(p1s[:], ps1[:], ACT.Copy)
                                nc.vector.tensor_tensor(att_sb[:, n * 512:(n + 1) * 512],
                                                        p1s[:], ps2[:], op=ALU.add)
                            nc.sync.dma_start(rsin[m * 128:(m + 1) * 128, :], att_sb[:])
                    rsout = dram.tile([MSH, D], BF16, tag="rsout")
                    nc.gpsimd.collective_compute("ReduceScatter", ALU.add,
                                                 replica_groups=RG, ins=[rsin[:]], outs=[rsout[:]])
                    with tc.tile_pool(name=f"res{l}", bufs=2) as rsp:
                        for m in range(4):
                            r16 = rsp.tile([128, D], BF16, tag="r16")
                            nc.sync.dma_start(r16[:], rsout[m * 128:(m + 1) * 128, :])
                            rf = rsp.tile([128, D], F32, tag="rf")
                            nc.vector.tensor_copy(rf[:], r16[:])
                            rf2 = rsp.tile([128, D], F32, tag="rf2")
                            nc.vector.tensor_tensor(rf2[:], rf[:], bo_r[:], op=ALU.add)
                            nc.vector.tensor_tensor(xc[:, m * D:(m + 1) * D],
                                                    xc[:, m * D:(m + 1) * D], rf2[:], op=ALU.add)

                    # === LN2 -> AG -> MLP -> RS -> residual ===
                    h216 = scr.tile([128, 4 * D], BF16, tag="h216")
                    with tc.tile_pool(name=f"ln{l}b", bufs=2) as lnp_:
                        layer_norm(lnp_, ln2w[:], ln2b[:],
                                   lambda m: xc[:, m * D:(m + 1) * D],
                                   lambda m: h216[:, m * D:(m + 1) * D])
                    agin2 = dram.tile([MSH, D], BF16, tag="agin2")
                    for m in range(4):
                        nc.sync.dma_start(agin2[m * 128:(m + 1) * 128, :], h216[:, m * D:(m + 1) * D])
                    agout2 = dram.tile([M, D], BF16, tag="agout2")
                    nc.gpsimd.collective_compute("AllGather", mybir.AluOpType.bypass,
                                                 replica_groups=RG, ins=[agin2[:]], outs=[agout2[:]])
                    rsin2 = dram.tile([M, D], BF16, tag="rsin2")
                    with tc.tile_pool(name=f"mlp{l}", bufs=3) as mp, \
                         tc.tile_pool(name=f"mps{l}", bufs=4, space=bass.MemorySpace.PSUM) as pm1, \
                         tc.tile_pool(name=f"mps2{l}", bufs=4, space=bass.MemorySpace.PSUM) as pm2:
                        for c in range(NCH):
                            hts = []
                            for k in range(NKT):
                                ht = mp.tile([128, 512], BF16, tag=f"h2t{k}")
                                nc.sync.dma_start_transpose(ht[:], agout2[c * 512:(c + 1) * 512, k * 128:(k + 1) * 128])
                                hts.append(ht)
                            gch = mp.tile([128, 4 * 512], BF16, tag="gch")
                            for f in range(4):
                                ps = pm1.tile([128, 512], F32, tag="psh1")
                                for k in range(NKT):
                                    nc.tensor.matmul(ps[:], w1[:, k * FSH + f * 128: k * FSH + (f + 1) * 128],
                                                     hts[k][:], start=(k == 0), stop=(k == NKT - 1))
                                nc.scalar.activation(gch[:, f * 512:(f + 1) * 512], ps[:], ACT.Gelu,
                                                     bias=b1t[:, f:f + 1])
                            for ms in range(4):
                                mo = mp.tile([128, D], BF16, tag="mo")
                                for n in range(2):
                                    ps = pm2.tile([128, 512], F32, tag="psh2")
                                    for f in range(4):
                                        nc.tensor.matmul(ps[:], gch[:, f * 512 + ms * 128: f * 512 + (ms + 1) * 128],
                                                         w2[:, f * D + n * 512: f * D + n * 512 + 512],
                                                         start=(f == 0), stop=(f == 3))
                                    nc.scalar.activation(mo[:, n * 512:(n + 1) * 512], ps[:], ACT.Copy)
                                nc.sync.dma_start(rsin2[c * 512 + ms * 128: c * 512 + (ms + 1) * 128, :], mo[:])
                    rsout2 = dram.tile([MSH, D], BF16, tag="rsout2")
                    nc.gpsimd.collective_compute("ReduceScatter", ALU.add,
                                                 replica_groups=RG, ins=[rsin2[:]], outs=[rsout2[:]])
                    with tc.tile_pool(name=f"res{l}b", bufs=2) as rsp:
                        for m in range(4):
                            r16 = rsp.tile([128, D], BF16, tag="r16b")
                            nc.sync.dma_start(r16[:], rsout2[m * 128:(m + 1) * 128, :])
                            rf = rsp.tile([128, D], F32, tag="rfb")
                            nc.vector.tensor_copy(rf[:], r16[:])
                            rf2 = rsp.tile([128, D], F32, tag="rfb2")
                            nc.vector.tensor_tensor(rf2[:], rf[:], b2_r[:], op=ALU.add)
                            nc.vector.tensor_tensor(xc[:, m * D:(m + 1) * D],
                                                    xc[:, m * D:(m + 1) * D], rf2[:], op=ALU.add)

            # === post-LN (after last layer) then final LN ===
            with tc.tile_pool(name="fin", bufs=1) as fin, \
                 tc.tile_pool(name="fins", bufs=2) as fscr:
                reps = []
                for j in range(4):
                    row = fscr.tile([1, D], F32, tag="lnfrow")
                    nc.sync.dma_start(row[:], LNF[j:j + 1, :])
                    r = fin.tile([128, D], F32, tag=f"repf{j}")
                    nc.gpsimd.partition_broadcast(r[:], row[:])
                    reps.append(r)
                postw, postb, lnfw, lnfb = reps
                xpost = fin.tile([128, 4 * D], F32, tag="xpost")
                with tc.tile_pool(name="lnpost", bufs=2) as lnp_:
                    layer_norm(lnp_, postw[:], postb[:],
                               lambda m: xc[:, m * D:(m + 1) * D],
                               lambda m: xpost[:, m * D:(m + 1) * D])
                xf16 = fin.tile([128, 4 * D], BF16, tag="xf16")
                with tc.tile_pool(name="lnfin", bufs=2) as lnp_:
                    layer_norm(lnp_, lnfw[:], lnfb[:],
                               lambda m: xpost[:, m * D:(m + 1) * D],
                               lambda m: xf16[:, m * D:(m + 1) * D])
                for m in range(4):
                    nc.sync.dma_start(XF[m * 128:(m + 1) * 128, :], xf16[:, m * D:(m + 1) * D])
                aginf = dram.tile([MSH, D], BF16, tag="aginf")
                for m in range(4):
                    nc.sync.dma_start(aginf[m * 128:(m + 1) * 128, :], xf16[:, m * D:(m + 1) * D])
                agoutf = dram.tile([M, D], BF16, tag="agoutf")
                nc.gpsimd.collective_compute("AllGather", mybir.AluOpType.bypass,
                                             replica_groups=RG, ins=[aginf[:]], outs=[agoutf[:]])

                # === vocab head: logits chunks + exp-sum ===
                se = fin.tile([128, NMT], F32, tag="se")
                with tc.tile_pool(name="head", bufs=3) as hp, \
                     tc.tile_pool(name="xfp", bufs=1) as xfp, \
                     tc.tile_pool(name="heads", bufs=4, space=bass.MemorySpace.PSUM) as php:
                    xfT = []
                    for k in range(NKT):
                        for c in range(NCH):
                            xt = xfp.tile([128, 512], BF16, tag=f"xfT{k}_{c}")
                            nc.sync.dma_start_transpose(xt[:], agoutf[c * 512:(c + 1) * 512, k * 128:(k + 1) * 128])
                            xfT.append(xt)  # index k*NCH+c
                    separts = fin.tile([128, NMT * NVC], F32, tag="separts")
                    for n in range(NVC):
                        hw = hp.tile([128, NKT * 512], BF16, tag="hw")
                        for k in range(NKT):
                            nc.sync.dma_start(hw[:, k * 512:(k + 1) * 512], HWT[k * 128:(k + 1) * 128, n * 512:(n + 1) * 512])
                        for m in range(NMT):
                            c, moff = m // 4, (m % 4) * 128
                            ps = php.tile([128, 512], F32, tag="pshead")
                            for k in range(NKT):
                                nc.tensor.matmul(ps[:], xfT[k * NCH + c][:, moff:moff + 128],
                                                 hw[:, k * 512:(k + 1) * 512],
                                                 start=(k == 0), stop=(k == NKT - 1))
                            esc = hp.tile([128, 512], BF16, tag="esc")
                            nc.scalar.activation(esc[:], ps[:], ACT.Exp,
                                                 accum_out=separts[:, m * NVC + n: m * NVC + n + 1])
                    for m in range(NMT):
                        nc.vector.tensor_reduce(se[:, m:m + 1], separts[:, m * NVC:(m + 1) * NVC],
                                                axis=AX.X, op=ALU.add)
                nc.sync.dma_start(SE, se[:])

    nc.compile()
    return nc


def _host_prep(inputs):
    f32 = np.float32
    tokens = np.asarray(inputs["tokens"]).reshape(M)
    targets = np.asarray(inputs["targets"]).reshape(M)
    word_emb = np.asarray(inputs["word_emb"], dtype=f32)
    pos_emb = np.asarray(inputs["pos_emb"], dtype=f32)
    x0 = (word_emb[tokens].reshape(B, S, D) + pos_emb[None, :, :]).reshape(M, D).astype(f32)

    inv = (1.0 / (10000.0 ** (np.arange(0, DH, 2, dtype=f32) / DH))).astype(f32)  # [32]
    s_idx = (np.arange(M) % S).astype(f32)
    fr = s_idx[None, :] * inv[:, None]          # [32, M]
    cos32 = np.cos(fr).astype(f32)
    sin32 = np.sin(fr).astype(f32)
    cos = np.concatenate([cos32, cos32], 0)      # [64, M]
    sinpm = np.concatenate([-sin32, sin32], 0)   # [64, M]

    per_core = []
    for c in range(NC):
        d = {}
        d["X0"] = np.ascontiguousarray(x0[c * MSH:(c + 1) * MSH])
        ds = slice(c * DSH, (c + 1) * DSH)
        fs = slice(c * FSH, (c + 1) * FSH)
        d["WQ"] = np.ascontiguousarray(np.asarray(inputs["wq"], f32)[:, :, ds]).astype(bf16)
        d["WK"] = np.ascontiguousarray(np.asarray(inputs["wk"], f32)[:, :, ds]).astype(bf16)
        d["WV"] = np.ascontiguousarray(np.asarray(inputs["wv"], f32)[:, :, ds]).astype(bf16)
        d["WO"] = np.ascontiguousarray(np.asarray(inputs["wo"], f32)[:, ds, :]).astype(bf16)
        d["W1"] = np.ascontiguousarray(np.asarray(inputs["w1"], f32)[:, :, fs]).astype(bf16)
        d["W2"] = np.ascontiguousarray(np.asarray(inputs["w2"], f32)[:, fs, :]).astype(bf16)
        d["BQ"] = np.ascontiguousarray(np.asarray(inputs["bq"], f32)[:, ds])
        d["BK"] = np.ascontiguousarray(np.asarray(inputs["bk"], f32)[:, ds])
        d["BV"] = np.ascontiguousarray(np.asarray(inputs["bv"], f32)[:, ds])
        d["B1"] = np.ascontiguousarray(np.asarray(inputs["b1"], f32)[:, fs])
        d["LNP"] = np.ascontiguousarray(np.stack([
            np.asarray(inputs["ln1_w"], f32), np.asarray(inputs["ln1_b"], f32),
            np.asarray(inputs["ln2_w"], f32), np.asarray(inputs["ln2_b"], f32),
            np.asarray(inputs["bo"], f32), np.asarray(inputs["b2"], f32)], axis=1))
        d["LNF"] = np.ascontiguousarray(np.stack([
            np.asarray(inputs["post_w"], f32), np.asarray(inputs["post_b"], f32),
            np.asarray(inputs["lnf_w"], f32), np.asarray(inputs["lnf_b"], f32)], axis=0))
        d["COS"] = cos.astype(bf16)
        d["SINPM"] = sinpm.astype(bf16)
        lo = c * PERV
        hi = min(lo + PERV, V)
        hw = np.zeros((D, VPAD), f32)
        hw[:, :hi - lo] = np.asarray(inputs["head_w"], f32)[:, lo:hi]
        d["HW"] = hw.astype(bf16)
        per_core.append(d)
    return per_core, targets


def _kernel_device(inputs):
    import time, sys
    from concourse import bass_utils
    t0 = time.time()
    if "nc" not in _CACHE:
        _CACHE["nc"] = _build()
    nc = _CACHE["nc"]
    t1 = time.time()
    per_core, targets = _host_prep(inputs)
    t2 = time.time()
    res = bass_utils.run_bass_kernel_spmd(nc, per_core, core_ids=list(range(NC)))
    t3 = time.time()
    print(f"[kernel] build={t1-t0:.2f}s prep={t2-t1:.2f}s run={t3-t2:.2f}s", file=sys.stderr, flush=True)

    # assemble x_f [M, D] from shards
    xf = np.concatenate([res.results[c]["XF"].astype(np.float32) for c in range(NC)], axis=0)
    # per-token sumexp across cores (minus padding contributions exp(0)=1)
    se_tot = np.zeros(M, np.float64)
    for c in range(NC):
        se = res.results[c]["SE"].astype(np.float64)       # [128, 32]
        pad = VPAD - (min((c + 1) * PERV, V) - c * PERV)
        se_tot += se.T.reshape(M) - pad
    head_w = np.asarray(inputs["head_w"], np.float32)
    picked = np.einsum("md,md->m", xf, head_w[:, targets].T.astype(np.float32))
    nll = np.log(se_tot) - picked
    return np.float32(nll.mean())


# ---------------------------------------------------------------------------
# Pure-numpy fallback (no device): used only if the Bass path raises.

def _np_ln(x, w, b):
    m = x.mean(-1, keepdims=True, dtype=np.float32)
    v = ((x - m) ** 2).mean(-1, keepdims=True, dtype=np.float32)
    return ((x - m) / np.sqrt(v + 1e-5) * w + b).astype(np.float32)


def _np_rope(x):
    dh = x.shape[-1]
    inv = 1.0 / (10000.0 ** (np.arange(0, dh, 2, dtype=np.float32) / dh))
    t = np.arange(x.shape[-2], dtype=np.float32)
    fr = t[:, None] * inv[None, :]
    emb = np.concatenate([fr, fr], axis=-1)
    cosv, sinv = np.cos(emb).astype(np.float32), np.sin(emb).astype(np.float32)
    half = dh // 2
    rot = np.concatenate([-x[..., half:], x[..., :half]], axis=-1)
    return (x * cosv + rot * sinv).astype(np.float32)


def _np_gelu(x):
    from scipy.special import erf
    return (x * 0.5 * (1.0 + erf(x / np.float32(np.sqrt(2.0))))).astype(np.float32)


def _kernel_numpy(inputs):
    f32 = lambda a: np.asarray(a, dtype=np.float32)
    tokens = np.asarray(inputs["tokens"]).reshape(M)
    targets = np.asarray(inputs["targets"]).reshape(M)
    x = (f32(inputs["word_emb"])[tokens].reshape(B, S, D)
         + f32(inputs["pos_emb"])[None]).reshape(M, D)
    mask = np.tril(np.ones((S, S), dtype=bool))
    scale = np.float32(1.0 / np.sqrt(DH))
    wq, wk, wv, wo = (f32(inputs[k]) for k in ("wq", "wk", "wv", "wo"))
    w1, w2 = f32(inputs["w1"]), f32(inputs["w2"])
    for i in range(L):
        h = _np_ln(x, f32(inputs["ln1_w"])[i], f32(inputs["ln1_b"])[i])
        q = (h @ wq[i] + f32(inputs["bq"])[i]).reshape(B, S, H, DH).transpose(0, 2, 1, 3)
        k = (h @ wk[i] + f32(inputs["bk"])[i]).reshape(B, S, H, DH).transpose(0, 2, 1, 3)
        v = (h @ wv[i] + f32(inputs["bv"])[i]).reshape(B, S, H, DH).transpose(0, 2, 1, 3)
        q, k = _np_rope(q), _np_rope(k)
        o = np.empty((B, H, S, DH), np.float32)
        for b_ in range(B):
            for h_ in range(H):
                sc = (q[b_, h_] @ k[b_, h_].T) * scale
                sc = np.where(mask, sc, np.float32(-1e9)).astype(np.float32)
                sc -= sc.max(-1, keepdims=True)
                e = np.exp(sc)
                o[b_, h_] = (e / e.sum(-1, keepdims=True, dtype=np.float32)) @ v[b_, h_]
        o = o.transpose(0, 2, 1, 3).reshape(M, D)
        x = (x + o @ wo[i] + f32(inputs["bo"])[i]).astype(np.float32)
        h2 = _np_ln(x, f32(inputs["ln2_w"])[i], f32(inputs["ln2_b"])[i])
        x = (x + _np_gelu(h2 @ w1[i] + f32(inputs["b1"])[i]) @ w2[i]
             + f32(inputs["b2"])[i]).astype(np.float32)
        if i == L - 1:
            x = _np_ln(x, f32(inputs["post_w"]), f32(inputs["post_b"]))
    x = _np_ln(x, f32(inputs["lnf_w"]), f32(inputs["lnf_b"]))
    head_w = f32(inputs["head_w"])
    nll = np.zeros(M, np.float64)
    for mstart in range(0, M, 512):
        logits = x[mstart:mstart + 512] @ head_w
        mx = logits.max(-1, keepdims=True)
        lse = mx[:, 0] + np.log(np.exp(logits - mx).sum(-1, dtype=np.float32))
        picked = logits[np.arange(512), targets[mstart:mstart + 512]]
        nll[mstart:mstart + 512] = lse - picked
    return np.float32(nll.mean())


def kernel(**inputs):
    try:
        return _kernel_device(inputs)
    except Exception:
        import traceback
        traceback.print_exc()
        return _kernel_numpy(inputs)
